# revision 6
# baseline (speedup 1.0000x reference)
"""Distributed Bass kernel for nn_Attention_32701880992127 on 8 TRN2 NeuronCores.

Sharding (tensor parallel over heads): core d owns q-heads {2d, 2d+1} and
kv-head d//2 (GQA consecutive-repeat mapping). wq/wk/wv are column-split,
wo is split along its OUTPUT dim so each core produces a distinct column
slice of the final output from the AllGathered attention features.

All matmuls run in bf16 (f32 PSUM accumulation); elementwise math stays f32.
Softmax needs no max-subtraction (qk-norm bounds the scores); the sink
correction folds into the denominator:
    out_h = (sum_k exp(s_qk) v_k) / (exp(sink_h) + sum_k exp(s_qk)).
Scores are computed transposed (ST[k, q]) so exp's output directly feeds the
PV matmul as the moving operand. The causal diagonal 512-block is processed
as 4 k-chunks with shrinking q-windows; only the 128x128 diagonal block of
each chunk needs a (shared triangular) mask, applied on the GpSimd engine.

Engine balance: PE does only matmuls; qT/kT transposes go through the XBAR
DMA path on the sync queue; qk-norm is one fused square+accum DVE op per
head plus reciprocal+Sqrt; rope for q0|q1|k runs merged in 4 wide DVE ops
(pair-swap via stride-2 APs, sin sign pre-folded into the host table).
All DRAM inputs are pre-tiled partition-major so every DMA moves >=2KB
contiguous runs (the DMA queues are packet-rate limited). Emission keeps the
PE FIFO free of collective-dependent work until all local attention is done.
"""
import numpy as np
import ml_dtypes

import concourse.mybir as mybir
import concourse.tile as tile
from concourse import bacc
from concourse.bass_utils import run_bass_kernel_spmd

dt = mybir.dt
AO = mybir.AluOpType
AF = mybir.ActivationFunctionType
BF16 = ml_dtypes.bfloat16

N_CORES = 8
S = 2048            # sequence length
D = 2048            # model dim
DH = 128            # head dim
HL = 2              # local q heads per core
NC = 16             # d-chunks of 128
NST = 16            # s-tiles of 128
QT = 512            # attention q tile
NQT = S // QT
KC = 128            # attention k chunk
EW = 384            # rope width: q0|q1|k
RMS_EPS = 1.1920929e-07


def build():
    nc = bacc.Bacc("TRN2", target_bir_lowering=False, debug=False, num_devices=N_CORES)

    # all inputs pre-tiled partition-major on the host: [p, ...] with long
    # contiguous per-partition runs
    xt = nc.dram_tensor("xt", [128, NST * NC * 128], dt.bfloat16,
                        kind="ExternalInput").ap()            # [p, st, c, s]
    wqkv = nc.dram_tensor("wqkv", [128, NC * 512], dt.bfloat16,
                          kind="ExternalInput").ap()          # [p, c, e]
    wot = nc.dram_tensor("wot", [128, NC * HL * DH], dt.bfloat16,
                         kind="ExternalInput").ap()           # [p, c, e]
    cbar = nc.dram_tensor("cbar", [128, NST * EW], dt.bfloat16,
                          kind="ExternalInput").ap()          # [p, st, e] cos for q0|q1|k
    sbar = nc.dram_tensor("sbar", [128, NST * EW], dt.bfloat16,
                          kind="ExternalInput").ap()          # sign-folded sin
    trimask = nc.dram_tensor("trimask", [KC, KC], dt.bfloat16, kind="ExternalInput").ap()
    esd = nc.dram_tensor("es", [128, HL], dt.float32, kind="ExternalInput").ap()
    y_out = nc.dram_tensor("y", [S, HL * DH], dt.float32, kind="ExternalOutput").ap()

    with tile.TileContext(nc) as tc:
        with (
            tc.tile_pool(name="const", bufs=1) as cp,
            tc.tile_pool(name="work", bufs=2) as wp,
            tc.tile_pool(name="psum", bufs=2, space="PSUM") as pp,
            tc.tile_pool(name="dram", bufs=1, space="DRAM") as dp,
        ):
            # ---- persistent tiles ----
            wqkv_sb = cp.tile([128, NC, 512], dt.bfloat16, tag="wqkv")
            xt_sb = cp.tile([128, NST, NC, 128], dt.bfloat16, tag="xt")
            wot_sb = cp.tile([128, NC, HL * DH], dt.bfloat16, tag="wot")
            cbar_sb = cp.tile([128, NST, EW], dt.bfloat16, tag="cbar")
            sbar_sb = cp.tile([128, NST, EW], dt.bfloat16, tag="sbar")
            tri_sb = cp.tile([128, KC], dt.bfloat16, tag="tri")
            es_sb = cp.tile([128, HL], dt.float32, tag="es")
            ones128 = cp.tile([128, 128], dt.bfloat16, tag="ones128")
            nc.vector.memset(ones128[:], 1.0)

            qT = cp.tile([128, HL, NST, 128], dt.bfloat16, tag="qT")  # [dh, h, st, s]
            kT = cp.tile([128, NST, 128], dt.bfloat16, tag="kT")      # [dh, st, s]
            v_sb = cp.tile([128, NST, DH], dt.bfloat16, tag="v")      # [s, st, dh]

            # ---- AllGather bounce buffers (one pair per q-group) ----
            ag_ins = [dp.tile([HL * 128, QT], dt.bfloat16, name=f"ag_in{i}")
                      for i in range(NQT)]
            ag_outs = [dp.tile([N_CORES * HL * 128, QT], dt.bfloat16, addr_space="Shared",
                               name=f"ag_out{i}") for i in range(NQT)]

            # ---- input DMA schedule ----
            # sync stays short early (it also runs the qT/kT XBAR transposes);
            # scalar carries most inputs, interleaved so each piece lands just
            # before its consumer.
            xts = xt.rearrange("p (st e) -> p st e", st=NST)
            cbr = cbar.rearrange("p (st e) -> p st e", st=NST)
            sbr = sbar.rearrange("p (st e) -> p st e", st=NST)
            wqr = wqkv.rearrange("p (c e) -> p c e", c=NC)

            nc.scalar.dma_start(xt_sb[:, 0, :, :], xts[:, 0, :])
            for g in range(4):
                nc.sync.dma_start(wqkv_sb[:, 4 * g:4 * g + 4, :], wqr[:, 4 * g:4 * g + 4, :])
            nc.scalar.dma_start(xt_sb[:, 1, :, :], xts[:, 1, :])
            nc.scalar.dma_start(cbar_sb[:, 0:4, :], cbr[:, 0:4, :])
            nc.scalar.dma_start(sbar_sb[:, 0:4, :], sbr[:, 0:4, :])
            nc.scalar.dma_start(es_sb[:], esd)
            nc.sync.dma_start(xt_sb[:, 2, :, :], xts[:, 2, :])
            nc.sync.dma_start(xt_sb[:, 3, :, :], xts[:, 3, :])
            nc.sync.dma_start(tri_sb[:], trimask)
            for g in range(1, 4):
                nc.scalar.dma_start(cbar_sb[:, 4 * g:4 * g + 4, :], cbr[:, 4 * g:4 * g + 4, :])
                nc.scalar.dma_start(sbar_sb[:, 4 * g:4 * g + 4, :], sbr[:, 4 * g:4 * g + 4, :])
            for st in range(4, NST):
                eng = nc.sync if st in (4, 6) else nc.scalar
                eng.dma_start(xt_sb[:, st, :, :], xts[:, st, :])
            nc.scalar.dma_start(wot_sb[:], wot.rearrange("p (c e) -> p c e", c=NC))

            def proj(st):
                mm = pp.tile([128, 512], dt.float32, tag="mm")  # q[0:256] | k[256:384] | v[384:512]
                for c in range(NC):
                    nc.tensor.matmul(mm[:], xt_sb[:, st, c, :], wqkv_sb[:, c, :],
                                     start=(c == 0), stop=(c == NC - 1))

                # evacuate PSUM: q|k to f32 SBUF, v to bf16
                qk = wp.tile([128, EW], dt.float32, tag="qk", bufs=4)
                nc.vector.tensor_copy(qk[:], mm[:, 0:EW])
                nc.vector.tensor_copy(v_sb[:, st, :], mm[:, EW:512])

                # qk-norm: ssq via fused square+accum on DVE
                ssq = wp.tile([128, 4], dt.float32, tag="ssq")
                scr = wp.tile([128, 128], dt.float32, tag="scr")
                for i in range(3):
                    nc.vector.scalar_tensor_tensor(
                        out=scr[:], in0=qk[:, i * DH:(i + 1) * DH], scalar=1.0,
                        in1=qk[:, i * DH:(i + 1) * DH], op0=AO.bypass, op1=AO.mult,
                        accum_out=ssq[:, i:i + 1])
                # q: rsqrt(mean+eps) = sqrt(1/(ssq/128+eps)); k: extra 1/sqrt(128)
                nc.vector.tensor_scalar(out=ssq[:, 0:2], in0=ssq[:, 0:2],
                                        scalar1=1.0 / DH, scalar2=RMS_EPS,
                                        op0=AO.mult, op1=AO.add)
                nc.vector.tensor_scalar(out=ssq[:, 2:3], in0=ssq[:, 2:3],
                                        scalar1=DH * RMS_EPS, scalar2=None, op0=AO.add)
                rcp = wp.tile([128, 4], dt.float32, tag="rcp")
                nc.vector.reciprocal_approx_fast(rcp[:, 0:3], ssq[:, 0:3])
                rs = wp.tile([128, 4], dt.float32, tag="rs")
                nc.scalar.activation(rs[:, 0:3], rcp[:, 0:3], AF.Sqrt)

                # merged rope for q0|q1|k: u = qk*cos; w = pairswap(qk)*(+-sin)
                u1 = wp.tile([128, EW], dt.float32, tag="u1")
                w = wp.tile([128, EW], dt.float32, tag="w")
                nc.vector.tensor_tensor(out=u1[:], in0=qk[:], in1=cbar_sb[:, st, :],
                                        op=AO.mult)
                nc.vector.tensor_tensor(out=w[:, 0:EW:2], in0=qk[:, 1:EW:2],
                                        in1=sbar_sb[:, st, 0:EW:2], op=AO.mult)
                nc.vector.tensor_tensor(out=w[:, 1:EW:2], in0=qk[:, 0:EW:2],
                                        in1=sbar_sb[:, st, 1:EW:2], op=AO.mult)
                nc.vector.tensor_add(out=u1[:], in0=u1[:], in1=w[:])
                qhat = wp.tile([128, HL * DH], dt.bfloat16, tag="qhat")
                khat = wp.tile([128, DH], dt.bfloat16, tag="khat")
                for h in range(HL):
                    nc.vector.tensor_scalar(out=qhat[:, h * DH:(h + 1) * DH],
                                            in0=u1[:, h * DH:(h + 1) * DH],
                                            scalar1=rs[:, h:h + 1], scalar2=None, op0=AO.mult)
                nc.vector.tensor_scalar(out=khat[:], in0=u1[:, 2 * DH:EW],
                                        scalar1=rs[:, 2:3], scalar2=None, op0=AO.mult)

                # XBAR DMA transposes -> qT / kT (off the PE, sync queue)
                for h in range(HL):
                    nc.sync.dma_start(qT[:, h, st, :], qhat[:, h * DH:(h + 1) * DH],
                                      transpose=True)
                nc.sync.dma_start(kT[:, st, :], khat[:], transpose=True)

            def attn_group(t):
                # tasks: 2t full-chunk pairs over q [0:512), then 2 diagonal
                # pairs with shrinking q windows (only each chunk's first 128
                # columns need the triangular mask)
                tasks = [[(2 * p, 0), (2 * p + 1, 0)] for p in range(2 * t)]
                tasks.append([(4 * t, 0), (4 * t + 1, 128)])
                tasks.append([(4 * t + 2, 256), (4 * t + 3, 384)])
                ntask = len(tasks)
                for h in range(HL):
                    lacc = pp.tile([128, QT], dt.float32, tag="lacc", bufs=1)
                    oacc = pp.tile([128, QT], dt.float32, tag="oacc", bufs=1)
                    pts = [None] * ntask

                    def emit_score(i):
                        stp = pp.tile([128, 1024], dt.float32, tag="stp")
                        pt = wp.tile([128, 1024], dt.bfloat16, tag="pt", bufs=4)
                        segs = []
                        col = 0
                        for c, qoff in tasks[i]:
                            wd = QT - qoff
                            nc.tensor.matmul(stp[:, col:col + wd], kT[:, c, :],
                                             qT[:, h, 4 * t + qoff // 128:4 * t + 4, :],
                                             start=True, stop=True)
                            segs.append((c, qoff, wd, col))
                            col += wd
                        nc.scalar.activation(pt[:, 0:col], stp[:, 0:col], AF.Exp)
                        if i >= 2 * t:  # diagonal task: mask each chunk's diag block
                            for c, qoff, wd, col0 in segs:
                                nc.gpsimd.tensor_tensor(out=pt[:, col0:col0 + KC],
                                                        in0=pt[:, col0:col0 + KC],
                                                        in1=tri_sb[:], op=AO.mult)
                        pts[i] = (pt, segs)

                    def emit_acc(i, last):
                        pt, segs = pts[i]
                        for j, (c, qoff, wd, col0) in enumerate(segs):
                            fl = (i == 0 and j == 0)
                            ll = last and j == len(segs) - 1
                            nc.tensor.matmul(lacc[:, qoff:QT], ones128[:],
                                             pt[:, col0:col0 + wd], start=fl, stop=ll)
                            nc.tensor.matmul(oacc[:, qoff:QT], v_sb[:, c, :],
                                             pt[:, col0:col0 + wd], start=fl, stop=ll)

                    emit_score(0)
                    for i in range(1, ntask):
                        emit_score(i)
                        emit_acc(i - 1, last=False)
                    emit_acc(ntask - 1, last=True)

                    # out = oacc / (lacc + exp(sink))
                    tmp = wp.tile([128, QT], dt.float32, tag="tmp")
                    nc.vector.tensor_scalar(out=tmp[:], in0=lacc[:],
                                            scalar1=es_sb[:, h:h + 1], scalar2=None,
                                            op0=AO.add)
                    rr = wp.tile([128, QT], dt.float32, tag="rr")
                    nc.vector.reciprocal_approx_fast(rr[:], tmp[:])
                    att = wp.tile([128, QT], dt.bfloat16, tag="att")
                    nc.vector.tensor_tensor(out=att[:], in0=oacc[:], in1=rr[:], op=AO.mult)
                    nc.scalar.dma_start(
                        ag_ins[t][:].rearrange("(h p) q -> p h q", p=128)[:, h, :], att[:])
                nc.gpsimd.collective_compute(
                    "AllGather", AO.bypass,
                    replica_groups=[list(range(N_CORES))],
                    ins=[ag_ins[t][:].opt()], outs=[ag_outs[t][:].opt()],
                )

            def wo_part(t):
                agr = ag_outs[t][:].rearrange("(c p) q -> p c q", p=128)
                aT = wp.tile([128, NC, QT], dt.bfloat16, tag="aT", bufs=2)
                nc.sync.dma_start(aT[:, 0:8, :], agr[:, 0:8, :])
                nc.sync.dma_start(aT[:, 8:16, :], agr[:, 8:16, :])
                for tt in range(QT // 128):
                    qsl = slice(tt * 128, (tt + 1) * 128)
                    yp = pp.tile([128, 512], dt.float32, tag="mm")
                    for c in range(NC):
                        nc.tensor.matmul(yp[:, 0:HL * DH], aT[:, c, qsl], wot_sb[:, c, :],
                                         start=(c == 0), stop=(c == NC - 1))
                    ysb = wp.tile([128, HL * DH], dt.float32, tag="ysb")
                    nc.vector.tensor_copy(ysb[:], yp[:, 0:HL * DH])
                    nc.sync.dma_start(y_out[t * QT + tt * 128:t * QT + (tt + 1) * 128, :],
                                      ysb[:])

            # ---- emission: all local work first, wo (collective-dependent) last ----
            for st in range(NST):
                proj(st)
                if st >= 4 and st % 4 == 0:
                    attn_group(st // 4 - 1)
            attn_group(NQT - 1)
            for t in range(NQT):
                wo_part(t)

    nc.compile()
    return nc


def prep_inputs(x, freqs_cis, wq, wk, wv, wo, sinks):
    """Host-side sharding/layout prep. Returns in_maps for the 8 cores.

    All tensors are pre-tiled partition-major ([p, ...]) so DMAs move
    long contiguous per-partition runs.
    """
    x2 = np.ascontiguousarray(np.asarray(x, np.float32).reshape(S, D))
    xt = x2.T.astype(BF16)                                    # [D, S] = [(c p), (st s)]
    xt_h = np.ascontiguousarray(
        xt.reshape(NC, 128, NST, 128).transpose(1, 2, 0, 3).reshape(128, NST * NC * 128))

    fc = np.asarray(freqs_cis, np.float32)
    cos, sin = fc[:, :, 0], fc[:, :, 1]
    c1 = np.repeat(cos, 2, axis=1)             # [S, 128] pair-interleaved
    s1 = np.repeat(sin, 2, axis=1)
    cbar = np.concatenate([c1, c1, c1], axis=1).astype(np.float32)   # [S, 384] q0|q1|k
    sbar = np.concatenate([s1, s1, s1], axis=1).astype(np.float32)
    sbar[:, 0::2] *= -1.0                      # even outputs get -sin
    cbar_h = np.ascontiguousarray(
        cbar.reshape(NST, 128, EW).transpose(1, 0, 2).reshape(128, NST * EW)).astype(BF16)
    sbar_h = np.ascontiguousarray(
        sbar.reshape(NST, 128, EW).transpose(1, 0, 2).reshape(128, NST * EW)).astype(BF16)

    kr = np.arange(KC)[:, None]
    qr = np.arange(KC)[None, :]
    trimask = (qr >= kr).astype(np.float32).astype(BF16)      # [128, 128]

    wq = np.asarray(wq, np.float32)
    wk = np.asarray(wk, np.float32)
    wv = np.asarray(wv, np.float32)
    wo = np.asarray(wo, np.float32)
    sinks = np.asarray(sinks, np.float32)

    in_maps = []
    for d in range(N_CORES):
        kv = d // 2
        es = np.exp(sinks[2 * d:2 * d + 2]).astype(np.float32)
        wqkv = np.concatenate([
            wq[d * 256:(d + 1) * 256, :].T,
            wk[kv * 128:(kv + 1) * 128, :].T,
            wv[kv * 128:(kv + 1) * 128, :].T,
        ], axis=1).astype(BF16)                               # [D, 512] = [(c p), e]
        wqkv_h = np.ascontiguousarray(
            wqkv.reshape(NC, 128, 512).transpose(1, 0, 2).reshape(128, NC * 512))
        wotd = np.ascontiguousarray(wo[d * 256:(d + 1) * 256, :].T).astype(BF16)
        wot_h = np.ascontiguousarray(
            wotd.reshape(NC, 128, HL * DH).transpose(1, 0, 2).reshape(128, NC * HL * DH))
        in_maps.append({
            "xt": xt_h,
            "wqkv": wqkv_h,
            "wot": wot_h,
            "cbar": cbar_h,
            "sbar": sbar_h,
            "trimask": trimask,
            "es": np.repeat(es[None, :], 128, axis=0).astype(np.float32),
        })
    return in_maps


_CACHED = {}


def kernel(x, freqs_cis, wq, wk, wv, wo, sinks):
    if "nc" not in _CACHED:
        _CACHED["nc"] = build()
    nc = _CACHED["nc"]
    in_maps = prep_inputs(x, freqs_cis, wq, wk, wv, wo, sinks)
    res = run_bass_kernel_spmd(nc, in_maps, list(range(N_CORES)), trace=False)
    y = np.concatenate([res.results[d]["y"] for d in range(N_CORES)], axis=1)
    return y.reshape(1, S, D).astype(np.float32)


# revision 7
# speedup vs baseline: 1.0551x; 1.0551x over previous
"""Distributed Bass kernel for nn_Attention_32701880992127 on 8 TRN2 NeuronCores.

Sharding (tensor parallel over heads): core d owns q-heads {2d, 2d+1} and
kv-head d//2 (GQA consecutive-repeat mapping). wq/wk/wv are column-split,
wo is split along its OUTPUT dim so each core produces a distinct column
slice of the final output from the AllGathered attention features.

All matmuls run in bf16 (f32 PSUM accumulation); elementwise math stays f32.
Softmax needs no max-subtraction (qk-norm bounds the scores); the sink
correction folds into the denominator:
    out_h = (sum_k exp(s_qk) v_k) / (exp(sink_h) + sum_k exp(s_qk)).
Scores are computed transposed (ST[k, q]) so exp's output directly feeds the
PV matmul as the moving operand. The causal diagonal 512-block is processed
as 4 k-chunks with shrinking q-windows; only the 128x128 diagonal block of
each chunk needs a (shared triangular) mask, applied on the GpSimd engine.

Engine balance: qT/kT transposes run on the PE (XBAR DMA transposes cost
~1.2us each of serial queue time); qk-norm is one fused square+accum DVE op per
head plus reciprocal+Sqrt; rope for q0|q1|k runs merged in 4 wide DVE ops
(pair-swap via stride-2 APs, sin sign pre-folded into the host table).
All DRAM inputs are pre-tiled partition-major so every DMA moves >=2KB
contiguous runs (the DMA queues are packet-rate limited). Emission keeps the
PE FIFO free of collective-dependent work until all local attention is done.
"""
import numpy as np
import ml_dtypes

import concourse.mybir as mybir
import concourse.tile as tile
from concourse import bacc
from concourse.bass_utils import run_bass_kernel_spmd
from concourse.masks import make_identity

dt = mybir.dt
AO = mybir.AluOpType
AF = mybir.ActivationFunctionType
BF16 = ml_dtypes.bfloat16

N_CORES = 8
S = 2048            # sequence length
D = 2048            # model dim
DH = 128            # head dim
HL = 2              # local q heads per core
NC = 16             # d-chunks of 128
NST = 16            # s-tiles of 128
QT = 512            # attention q tile
NQT = S // QT
KC = 128            # attention k chunk
EW = 384            # rope width: q0|q1|k
RMS_EPS = 1.1920929e-07


def build():
    nc = bacc.Bacc("TRN2", target_bir_lowering=False, debug=False, num_devices=N_CORES)

    # all inputs pre-tiled partition-major on the host: [p, ...] with long
    # contiguous per-partition runs
    xt = nc.dram_tensor("xt", [128, NST * NC * 128], dt.bfloat16,
                        kind="ExternalInput").ap()            # [p, st, c, s]
    wqkv = nc.dram_tensor("wqkv", [128, NC * 512], dt.bfloat16,
                          kind="ExternalInput").ap()          # [p, c, e]
    wot = nc.dram_tensor("wot", [128, NC * HL * DH], dt.bfloat16,
                         kind="ExternalInput").ap()           # [p, c, e]
    cbar = nc.dram_tensor("cbar", [128, NST * EW], dt.bfloat16,
                          kind="ExternalInput").ap()          # [p, st, e] cos for q0|q1|k
    sbar = nc.dram_tensor("sbar", [128, NST * EW], dt.bfloat16,
                          kind="ExternalInput").ap()          # sign-folded sin
    trimask = nc.dram_tensor("trimask", [KC, KC], dt.bfloat16, kind="ExternalInput").ap()
    esd = nc.dram_tensor("es", [128, HL], dt.float32, kind="ExternalInput").ap()
    y_out = nc.dram_tensor("y", [S, HL * DH], dt.float32, kind="ExternalOutput").ap()

    with tile.TileContext(nc) as tc:
        with (
            tc.tile_pool(name="const", bufs=1) as cp,
            tc.tile_pool(name="work", bufs=2) as wp,
            tc.tile_pool(name="psum", bufs=2, space="PSUM") as pp,
            tc.tile_pool(name="dram", bufs=1, space="DRAM") as dp,
        ):
            # ---- persistent tiles ----
            wqkv_sb = cp.tile([128, NC, 512], dt.bfloat16, tag="wqkv")
            xt_sb = cp.tile([128, NST, NC, 128], dt.bfloat16, tag="xt")
            wot_sb = cp.tile([128, NC, HL * DH], dt.bfloat16, tag="wot")
            cbar_sb = cp.tile([128, NST, EW], dt.bfloat16, tag="cbar")
            sbar_sb = cp.tile([128, NST, EW], dt.bfloat16, tag="sbar")
            tri_sb = cp.tile([128, KC], dt.bfloat16, tag="tri")
            es_sb = cp.tile([128, HL], dt.float32, tag="es")
            ones128 = cp.tile([128, 128], dt.bfloat16, tag="ones128")
            nc.vector.memset(ones128[:], 1.0)
            ident = cp.tile([128, 128], dt.bfloat16, tag="ident")
            make_identity(nc, ident[:])

            qT = cp.tile([128, HL, NST, 128], dt.bfloat16, tag="qT")  # [dh, h, st, s]
            kT = cp.tile([128, NST, 128], dt.bfloat16, tag="kT")      # [dh, st, s]
            v_sb = cp.tile([128, NST, DH], dt.bfloat16, tag="v")      # [s, st, dh]

            # ---- AllGather bounce buffers (one pair per q-group) ----
            ag_ins = [dp.tile([HL * 128, QT], dt.bfloat16, name=f"ag_in{i}")
                      for i in range(NQT)]
            ag_outs = [dp.tile([N_CORES * HL * 128, QT], dt.bfloat16, addr_space="Shared",
                               name=f"ag_out{i}") for i in range(NQT)]

            # ---- input DMA schedule ----
            # sync stays short early (it also runs the qT/kT XBAR transposes);
            # scalar carries most inputs, interleaved so each piece lands just
            # before its consumer.
            xts = xt.rearrange("p (st e) -> p st e", st=NST)
            cbr = cbar.rearrange("p (st e) -> p st e", st=NST)
            sbr = sbar.rearrange("p (st e) -> p st e", st=NST)
            wqr = wqkv.rearrange("p (c e) -> p c e", c=NC)

            nc.scalar.dma_start(xt_sb[:, 0, :, :], xts[:, 0, :])
            for g in range(4):
                nc.sync.dma_start(wqkv_sb[:, 4 * g:4 * g + 4, :], wqr[:, 4 * g:4 * g + 4, :])
            nc.scalar.dma_start(xt_sb[:, 1, :, :], xts[:, 1, :])
            nc.scalar.dma_start(cbar_sb[:, 0:4, :], cbr[:, 0:4, :])
            nc.scalar.dma_start(sbar_sb[:, 0:4, :], sbr[:, 0:4, :])
            nc.scalar.dma_start(es_sb[:], esd)
            nc.sync.dma_start(xt_sb[:, 2, :, :], xts[:, 2, :])
            nc.sync.dma_start(xt_sb[:, 3, :, :], xts[:, 3, :])
            nc.sync.dma_start(tri_sb[:], trimask)
            for g in range(1, 4):
                nc.scalar.dma_start(cbar_sb[:, 4 * g:4 * g + 4, :], cbr[:, 4 * g:4 * g + 4, :])
                nc.scalar.dma_start(sbar_sb[:, 4 * g:4 * g + 4, :], sbr[:, 4 * g:4 * g + 4, :])
            for st in range(4, NST):
                eng = nc.sync if st in (4, 6) else nc.scalar
                eng.dma_start(xt_sb[:, st, :, :], xts[:, st, :])
            nc.scalar.dma_start(wot_sb[:], wot.rearrange("p (c e) -> p c e", c=NC))

            def proj(st):
                mm = pp.tile([128, 512], dt.float32, tag="mm")  # q[0:256] | k[256:384] | v[384:512]
                for c in range(NC):
                    nc.tensor.matmul(mm[:], xt_sb[:, st, c, :], wqkv_sb[:, c, :],
                                     start=(c == 0), stop=(c == NC - 1))

                # evacuate PSUM: q|k to f32 SBUF, v to bf16
                qk = wp.tile([128, EW], dt.float32, tag="qk", bufs=4)
                nc.vector.tensor_copy(qk[:], mm[:, 0:EW])
                nc.vector.tensor_copy(v_sb[:, st, :], mm[:, EW:512])

                # qk-norm: ssq via fused square+accum on DVE
                ssq = wp.tile([128, 4], dt.float32, tag="ssq")
                scr = wp.tile([128, 128], dt.float32, tag="scr")
                for i in range(3):
                    nc.vector.scalar_tensor_tensor(
                        out=scr[:], in0=qk[:, i * DH:(i + 1) * DH], scalar=1.0,
                        in1=qk[:, i * DH:(i + 1) * DH], op0=AO.bypass, op1=AO.mult,
                        accum_out=ssq[:, i:i + 1])
                # q: rsqrt(mean+eps) = sqrt(1/(ssq/128+eps)); k: extra 1/sqrt(128)
                nc.vector.tensor_scalar(out=ssq[:, 0:2], in0=ssq[:, 0:2],
                                        scalar1=1.0 / DH, scalar2=RMS_EPS,
                                        op0=AO.mult, op1=AO.add)
                nc.vector.tensor_scalar(out=ssq[:, 2:3], in0=ssq[:, 2:3],
                                        scalar1=DH * RMS_EPS, scalar2=None, op0=AO.add)
                rcp = wp.tile([128, 4], dt.float32, tag="rcp")
                nc.vector.reciprocal_approx_fast(rcp[:, 0:3], ssq[:, 0:3])
                rs = wp.tile([128, 4], dt.float32, tag="rs")
                nc.scalar.activation(rs[:, 0:3], rcp[:, 0:3], AF.Sqrt)

                # merged rope for q0|q1|k: u = qk*cos; w = pairswap(qk)*(+-sin)
                u1 = wp.tile([128, EW], dt.float32, tag="u1")
                w = wp.tile([128, EW], dt.float32, tag="w")
                nc.vector.tensor_tensor(out=u1[:], in0=qk[:], in1=cbar_sb[:, st, :],
                                        op=AO.mult)
                nc.vector.tensor_tensor(out=w[:, 0:EW:2], in0=qk[:, 1:EW:2],
                                        in1=sbar_sb[:, st, 0:EW:2], op=AO.mult)
                nc.vector.tensor_tensor(out=w[:, 1:EW:2], in0=qk[:, 0:EW:2],
                                        in1=sbar_sb[:, st, 1:EW:2], op=AO.mult)
                nc.vector.tensor_add(out=u1[:], in0=u1[:], in1=w[:])
                qhat = wp.tile([128, HL * DH], dt.bfloat16, tag="qhat")
                khat = wp.tile([128, DH], dt.bfloat16, tag="khat")
                for h in range(HL):
                    nc.vector.tensor_scalar(out=qhat[:, h * DH:(h + 1) * DH],
                                            in0=u1[:, h * DH:(h + 1) * DH],
                                            scalar1=rs[:, h:h + 1], scalar2=None, op0=AO.mult)
                nc.vector.tensor_scalar(out=khat[:], in0=u1[:, 2 * DH:EW],
                                        scalar1=rs[:, 2:3], scalar2=None, op0=AO.mult)

                # PE transposes -> qT / kT (PSUM copies on ACT)
                for h in range(HL):
                    tp = pp.tile([128, 128], dt.bfloat16, tag="tp")
                    nc.tensor.transpose(tp[:], qhat[:, h * DH:(h + 1) * DH], ident[:])
                    nc.scalar.copy(qT[:, h, st, :], tp[:])
                tpk = pp.tile([128, 128], dt.bfloat16, tag="tp")
                nc.tensor.transpose(tpk[:], khat[:], ident[:])
                nc.scalar.copy(kT[:, st, :], tpk[:])

            def attn_group(t):
                # chunk i: i < 4t -> full k-chunk c=i over q cols [0:512)
                #          i >= 4t -> diagonal chunk c=4t+j over q cols [128j:512)
                nch = 4 * t + 4
                for h in range(HL):
                    lacc = pp.tile([128, QT], dt.float32, tag="lacc", bufs=1)
                    oacc = pp.tile([128, QT], dt.float32, tag="oacc", bufs=1)
                    pts = [None] * nch

                    def chunk_info(i):
                        if i < 4 * t:
                            return i, 0
                        j = i - 4 * t
                        return 4 * t + j, 128 * j

                    def emit_score(i):
                        c, qoff = chunk_info(i)
                        wd = QT - qoff
                        stp = pp.tile([128, QT], dt.float32, tag="stp")
                        nc.tensor.matmul(stp[:, 0:wd], kT[:, c, :],
                                         qT[:, h, 4 * t + qoff // 128:4 * t + 4, :],
                                         start=True, stop=True)
                        pt = wp.tile([128, QT], dt.bfloat16, tag="pt", bufs=4)
                        nc.scalar.activation(pt[:, 0:wd], stp[:, 0:wd], AF.Exp)
                        if i >= 4 * t:
                            nc.gpsimd.tensor_tensor(out=pt[:, 0:KC], in0=pt[:, 0:KC],
                                                    in1=tri_sb[:], op=AO.mult)
                        pts[i] = (pt, c, qoff, wd)

                    def emit_acc(i, last):
                        pt, c, qoff, wd = pts[i]
                        nc.tensor.matmul(lacc[:, qoff:QT], ones128[:], pt[:, 0:wd],
                                         start=(i == 0), stop=last)
                        nc.tensor.matmul(oacc[:, qoff:QT], v_sb[:, c, :], pt[:, 0:wd],
                                         start=(i == 0), stop=last)

                    emit_score(0)
                    for i in range(1, nch):
                        emit_score(i)
                        emit_acc(i - 1, last=False)
                    emit_acc(nch - 1, last=True)

                    # out = oacc / (lacc + exp(sink))
                    tmp = wp.tile([128, QT], dt.float32, tag="tmp")
                    nc.vector.tensor_scalar(out=tmp[:], in0=lacc[:],
                                            scalar1=es_sb[:, h:h + 1], scalar2=None,
                                            op0=AO.add)
                    rr = wp.tile([128, QT], dt.float32, tag="rr")
                    nc.vector.reciprocal_approx_fast(rr[:], tmp[:])
                    att = wp.tile([128, QT], dt.bfloat16, tag="att")
                    nc.vector.tensor_tensor(out=att[:], in0=oacc[:], in1=rr[:], op=AO.mult)
                    nc.scalar.dma_start(
                        ag_ins[t][:].rearrange("(h p) q -> p h q", p=128)[:, h, :], att[:])
                nc.gpsimd.collective_compute(
                    "AllGather", AO.bypass,
                    replica_groups=[list(range(N_CORES))],
                    ins=[ag_ins[t][:].opt()], outs=[ag_outs[t][:].opt()],
                )

            def wo_part(t):
                agr = ag_outs[t][:].rearrange("(c p) q -> p c q", p=128)
                aT = wp.tile([128, NC, QT], dt.bfloat16, tag="aT", bufs=2)
                nc.sync.dma_start(aT[:, 0:8, :], agr[:, 0:8, :])
                nc.sync.dma_start(aT[:, 8:16, :], agr[:, 8:16, :])
                for tt in range(QT // 128):
                    qsl = slice(tt * 128, (tt + 1) * 128)
                    yp = pp.tile([128, 512], dt.float32, tag="mm")
                    for c in range(NC):
                        nc.tensor.matmul(yp[:, 0:HL * DH], aT[:, c, qsl], wot_sb[:, c, :],
                                         start=(c == 0), stop=(c == NC - 1))
                    ysb = wp.tile([128, HL * DH], dt.float32, tag="ysb")
                    nc.vector.tensor_copy(ysb[:], yp[:, 0:HL * DH])
                    nc.sync.dma_start(y_out[t * QT + tt * 128:t * QT + (tt + 1) * 128, :],
                                      ysb[:])

            # ---- emission: all local work first, wo (collective-dependent) last ----
            for st in range(NST):
                proj(st)
                if st >= 4 and st % 4 == 0:
                    attn_group(st // 4 - 1)
            attn_group(NQT - 1)
            for t in range(NQT):
                wo_part(t)

    nc.compile()
    return nc


def prep_inputs(x, freqs_cis, wq, wk, wv, wo, sinks):
    """Host-side sharding/layout prep. Returns in_maps for the 8 cores.

    All tensors are pre-tiled partition-major ([p, ...]) so DMAs move
    long contiguous per-partition runs.
    """
    x2 = np.ascontiguousarray(np.asarray(x, np.float32).reshape(S, D))
    xt = x2.T.astype(BF16)                                    # [D, S] = [(c p), (st s)]
    xt_h = np.ascontiguousarray(
        xt.reshape(NC, 128, NST, 128).transpose(1, 2, 0, 3).reshape(128, NST * NC * 128))

    fc = np.asarray(freqs_cis, np.float32)
    cos, sin = fc[:, :, 0], fc[:, :, 1]
    c1 = np.repeat(cos, 2, axis=1)             # [S, 128] pair-interleaved
    s1 = np.repeat(sin, 2, axis=1)
    cbar = np.concatenate([c1, c1, c1], axis=1).astype(np.float32)   # [S, 384] q0|q1|k
    sbar = np.concatenate([s1, s1, s1], axis=1).astype(np.float32)
    sbar[:, 0::2] *= -1.0                      # even outputs get -sin
    cbar_h = np.ascontiguousarray(
        cbar.reshape(NST, 128, EW).transpose(1, 0, 2).reshape(128, NST * EW)).astype(BF16)
    sbar_h = np.ascontiguousarray(
        sbar.reshape(NST, 128, EW).transpose(1, 0, 2).reshape(128, NST * EW)).astype(BF16)

    kr = np.arange(KC)[:, None]
    qr = np.arange(KC)[None, :]
    trimask = (qr >= kr).astype(np.float32).astype(BF16)      # [128, 128]

    wq = np.asarray(wq, np.float32)
    wk = np.asarray(wk, np.float32)
    wv = np.asarray(wv, np.float32)
    wo = np.asarray(wo, np.float32)
    sinks = np.asarray(sinks, np.float32)

    in_maps = []
    for d in range(N_CORES):
        kv = d // 2
        es = np.exp(sinks[2 * d:2 * d + 2]).astype(np.float32)
        wqkv = np.concatenate([
            wq[d * 256:(d + 1) * 256, :].T,
            wk[kv * 128:(kv + 1) * 128, :].T,
            wv[kv * 128:(kv + 1) * 128, :].T,
        ], axis=1).astype(BF16)                               # [D, 512] = [(c p), e]
        wqkv_h = np.ascontiguousarray(
            wqkv.reshape(NC, 128, 512).transpose(1, 0, 2).reshape(128, NC * 512))
        wotd = np.ascontiguousarray(wo[d * 256:(d + 1) * 256, :].T).astype(BF16)
        wot_h = np.ascontiguousarray(
            wotd.reshape(NC, 128, HL * DH).transpose(1, 0, 2).reshape(128, NC * HL * DH))
        in_maps.append({
            "xt": xt_h,
            "wqkv": wqkv_h,
            "wot": wot_h,
            "cbar": cbar_h,
            "sbar": sbar_h,
            "trimask": trimask,
            "es": np.repeat(es[None, :], 128, axis=0).astype(np.float32),
        })
    return in_maps


_CACHED = {}


def kernel(x, freqs_cis, wq, wk, wv, wo, sinks):
    if "nc" not in _CACHED:
        _CACHED["nc"] = build()
    nc = _CACHED["nc"]
    in_maps = prep_inputs(x, freqs_cis, wq, wk, wv, wo, sinks)
    res = run_bass_kernel_spmd(nc, in_maps, list(range(N_CORES)), trace=False)
    y = np.concatenate([res.results[d]["y"] for d in range(N_CORES)], axis=1)
    return y.reshape(1, S, D).astype(np.float32)


# revision 8
# speedup vs baseline: 1.1160x; 1.0578x over previous
"""Distributed Bass kernel for nn_Attention_32701880992127 on 8 TRN2 NeuronCores.

Sharding (tensor parallel over heads): core d owns q-heads {2d, 2d+1} and
kv-head d//2 (GQA consecutive-repeat mapping). wq/wk/wv are column-split,
wo is split along its OUTPUT dim so each core produces a distinct column
slice of the final output from the AllGathered attention features.

All matmuls run in bf16 (f32 PSUM accumulation); elementwise math stays f32.
Softmax needs no max-subtraction (qk-norm bounds the scores); the sink
correction folds into the denominator:
    out_h = (sum_k exp(s_qk) v_k) / (exp(sink_h) + sum_k exp(s_qk)).
Scores are computed transposed (ST[k, q]) so exp's output directly feeds the
PV matmul as the moving operand. The causal diagonal 512-block is processed
as 4 k-chunks with shrinking q-windows; only the 128x128 diagonal block of
each chunk needs a (shared triangular) mask, applied on the GpSimd engine.

Engine balance: qT/kT transposes run on the PE (XBAR DMA transposes cost
~1.2us each of serial queue time); qk-norm is one fused square+accum DVE op per
head plus reciprocal+Sqrt; rope for q0|q1|k runs merged in 4 wide DVE ops
(pair-swap via stride-2 APs, sin sign pre-folded into the host table).
All DRAM inputs are pre-tiled partition-major so every DMA moves >=2KB
contiguous runs (the DMA queues are packet-rate limited). Emission keeps the
PE FIFO free of collective-dependent work until all local attention is done.
"""
import numpy as np
import ml_dtypes

import concourse.mybir as mybir
import concourse.tile as tile
from concourse import bacc
from concourse.bass_utils import run_bass_kernel_spmd
from concourse.masks import make_identity

dt = mybir.dt
AO = mybir.AluOpType
AF = mybir.ActivationFunctionType
BF16 = ml_dtypes.bfloat16

N_CORES = 8
S = 2048            # sequence length
D = 2048            # model dim
DH = 128            # head dim
HL = 2              # local q heads per core
NC = 16             # d-chunks of 128
NST = 16            # s-tiles of 128
QT = 512            # attention q tile
NQT = S // QT
KC = 128            # attention k chunk
EW = 384            # rope width: q0|q1|k
RMS_EPS = 1.1920929e-07


def build():
    nc = bacc.Bacc("TRN2", target_bir_lowering=False, debug=False, num_devices=N_CORES)

    # all inputs pre-tiled partition-major on the host: [p, ...] with long
    # contiguous per-partition runs
    xt = nc.dram_tensor("xt", [128, NST * NC * 128], dt.bfloat16,
                        kind="ExternalInput").ap()            # [p, st, c, s]
    wqkv = nc.dram_tensor("wqkv", [128, NC * 512], dt.bfloat16,
                          kind="ExternalInput").ap()          # [p, c, e]
    wot = nc.dram_tensor("wot", [128, NC * HL * DH], dt.bfloat16,
                         kind="ExternalInput").ap()           # [p, c, e]
    cbar = nc.dram_tensor("cbar", [128, NST * EW], dt.bfloat16,
                          kind="ExternalInput").ap()          # [p, st, e] cos for q0|q1|k
    sbar = nc.dram_tensor("sbar", [128, NST * EW], dt.bfloat16,
                          kind="ExternalInput").ap()          # sign-folded sin
    trimask = nc.dram_tensor("trimask", [KC, KC], dt.bfloat16, kind="ExternalInput").ap()
    esd = nc.dram_tensor("es", [128, HL], dt.float32, kind="ExternalInput").ap()
    y_out = nc.dram_tensor("y", [S, HL * DH], dt.float32, kind="ExternalOutput").ap()

    with tile.TileContext(nc) as tc:
        with (
            tc.tile_pool(name="const", bufs=1) as cp,
            tc.tile_pool(name="work", bufs=2) as wp,
            tc.tile_pool(name="psum", bufs=2, space="PSUM") as pp,
            tc.tile_pool(name="dram", bufs=1, space="DRAM") as dp,
        ):
            # ---- persistent tiles ----
            wqkv_sb = cp.tile([128, NC, 512], dt.bfloat16, tag="wqkv")
            xt_sb = cp.tile([128, NST, NC, 128], dt.bfloat16, tag="xt")
            wot_sb = cp.tile([128, NC, HL * DH], dt.bfloat16, tag="wot")
            cbar_sb = cp.tile([128, NST, EW], dt.bfloat16, tag="cbar")
            sbar_sb = cp.tile([128, NST, EW], dt.bfloat16, tag="sbar")
            tri_sb = cp.tile([128, KC], dt.bfloat16, tag="tri")
            es_sb = cp.tile([128, HL], dt.float32, tag="es")
            ones128 = cp.tile([128, 128], dt.bfloat16, tag="ones128")
            nc.vector.memset(ones128[:], 1.0)
            ident = cp.tile([128, 128], dt.bfloat16, tag="ident")
            make_identity(nc, ident[:])

            qT = cp.tile([128, HL, NST, 128], dt.bfloat16, tag="qT")  # [dh, h, st, s]
            kT = cp.tile([128, NST, 128], dt.bfloat16, tag="kT")      # [dh, st, s]
            v_sb = cp.tile([128, NST, DH], dt.bfloat16, tag="v")      # [s, st, dh]

            # ---- AllGather bounce buffers (one pair per q-group) ----
            ag_ins = [dp.tile([HL * 128, QT], dt.bfloat16, name=f"ag_in{i}")
                      for i in range(NQT)]
            ag_outs = [dp.tile([N_CORES * HL * 128, QT], dt.bfloat16, addr_space="Shared",
                               name=f"ag_out{i}") for i in range(NQT)]

            # ---- input DMA schedule ----
            # sync stays short early (it also runs the qT/kT XBAR transposes);
            # scalar carries most inputs, interleaved so each piece lands just
            # before its consumer.
            xts = xt.rearrange("p (st e) -> p st e", st=NST)
            cbr = cbar.rearrange("p (st e) -> p st e", st=NST)
            sbr = sbar.rearrange("p (st e) -> p st e", st=NST)
            wqr = wqkv.rearrange("p (c e) -> p c e", c=NC)

            nc.scalar.dma_start(xt_sb[:, 0, :, :], xts[:, 0, :])
            for g in range(4):
                nc.sync.dma_start(wqkv_sb[:, 4 * g:4 * g + 4, :], wqr[:, 4 * g:4 * g + 4, :])
            nc.scalar.dma_start(xt_sb[:, 1, :, :], xts[:, 1, :])
            nc.scalar.dma_start(cbar_sb[:, 0:4, :], cbr[:, 0:4, :])
            nc.scalar.dma_start(sbar_sb[:, 0:4, :], sbr[:, 0:4, :])
            nc.scalar.dma_start(es_sb[:], esd)
            nc.sync.dma_start(xt_sb[:, 2, :, :], xts[:, 2, :])
            nc.sync.dma_start(xt_sb[:, 3, :, :], xts[:, 3, :])
            nc.sync.dma_start(tri_sb[:], trimask)
            for g in range(1, 4):
                nc.scalar.dma_start(cbar_sb[:, 4 * g:4 * g + 4, :], cbr[:, 4 * g:4 * g + 4, :])
                nc.scalar.dma_start(sbar_sb[:, 4 * g:4 * g + 4, :], sbr[:, 4 * g:4 * g + 4, :])
            for st in range(4, NST):
                eng = nc.sync if st in (4, 6) else nc.scalar
                eng.dma_start(xt_sb[:, st, :, :], xts[:, st, :])
            nc.scalar.dma_start(wot_sb[:], wot.rearrange("p (c e) -> p c e", c=NC))

            def proj(st):
                mm = pp.tile([128, 512], dt.float32, tag="mm")  # q[0:256] | k[256:384] | v[384:512]
                for c in range(NC):
                    nc.tensor.matmul(mm[:], xt_sb[:, st, c, :], wqkv_sb[:, c, :],
                                     start=(c == 0), stop=(c == NC - 1))

                # evacuate PSUM: q|k to f32 SBUF, v to bf16
                qk = wp.tile([128, EW], dt.float32, tag="qk", bufs=4)
                nc.vector.tensor_copy(qk[:], mm[:, 0:EW])
                nc.vector.tensor_copy(v_sb[:, st, :], mm[:, EW:512])

                # qk-norm: ssq via fused square+accum on DVE
                ssq = wp.tile([128, 4], dt.float32, tag="ssq")
                scr = wp.tile([128, 128], dt.float32, tag="scr")
                for i in range(3):
                    nc.vector.scalar_tensor_tensor(
                        out=scr[:], in0=qk[:, i * DH:(i + 1) * DH], scalar=1.0,
                        in1=qk[:, i * DH:(i + 1) * DH], op0=AO.bypass, op1=AO.mult,
                        accum_out=ssq[:, i:i + 1])
                # q: rsqrt(mean+eps) = sqrt(1/(ssq/128+eps)); k: extra 1/sqrt(128)
                nc.vector.tensor_scalar(out=ssq[:, 0:2], in0=ssq[:, 0:2],
                                        scalar1=1.0 / DH, scalar2=RMS_EPS,
                                        op0=AO.mult, op1=AO.add)
                nc.vector.tensor_scalar(out=ssq[:, 2:3], in0=ssq[:, 2:3],
                                        scalar1=DH * RMS_EPS, scalar2=None, op0=AO.add)
                rcp = wp.tile([128, 4], dt.float32, tag="rcp")
                nc.vector.reciprocal_approx_fast(rcp[:, 0:3], ssq[:, 0:3])
                rs = wp.tile([128, 4], dt.float32, tag="rs")
                nc.scalar.activation(rs[:, 0:3], rcp[:, 0:3], AF.Sqrt)

                # merged rope for q0|q1|k: u = qk*cos; w = pairswap(qk)*(+-sin)
                u1 = wp.tile([128, EW], dt.float32, tag="u1")
                w = wp.tile([128, EW], dt.float32, tag="w")
                nc.vector.tensor_tensor(out=u1[:], in0=qk[:], in1=cbar_sb[:, st, :],
                                        op=AO.mult)
                nc.vector.tensor_tensor(out=w[:, 0:EW:2], in0=qk[:, 1:EW:2],
                                        in1=sbar_sb[:, st, 0:EW:2], op=AO.mult)
                nc.vector.tensor_tensor(out=w[:, 1:EW:2], in0=qk[:, 0:EW:2],
                                        in1=sbar_sb[:, st, 1:EW:2], op=AO.mult)
                nc.vector.tensor_add(out=u1[:], in0=u1[:], in1=w[:])
                qhat = wp.tile([128, HL * DH], dt.bfloat16, tag="qhat")
                khat = wp.tile([128, DH], dt.bfloat16, tag="khat")
                for h in range(HL):
                    nc.vector.tensor_scalar(out=qhat[:, h * DH:(h + 1) * DH],
                                            in0=u1[:, h * DH:(h + 1) * DH],
                                            scalar1=rs[:, h:h + 1], scalar2=None, op0=AO.mult)
                nc.vector.tensor_scalar(out=khat[:], in0=u1[:, 2 * DH:EW],
                                        scalar1=rs[:, 2:3], scalar2=None, op0=AO.mult)

                # PE transposes -> qT / kT (PSUM copies on ACT)
                for h in range(HL):
                    tp = pp.tile([128, 128], dt.bfloat16, tag="tp")
                    nc.tensor.transpose(tp[:], qhat[:, h * DH:(h + 1) * DH], ident[:])
                    nc.scalar.copy(qT[:, h, st, :], tp[:])
                tpk = pp.tile([128, 128], dt.bfloat16, tag="tp")
                nc.tensor.transpose(tpk[:], khat[:], ident[:])
                nc.scalar.copy(kT[:, st, :], tpk[:])

            def attn_group(t):
                # chunk i: i < 4t -> full k-chunk c=i over q cols [0:512)
                #          i >= 4t -> diagonal chunk c=4t+j over q cols [128j:512)
                nch = 4 * t + 4
                for h in range(HL):
                    lacc = pp.tile([128, QT], dt.float32, tag="lacc", bufs=1)
                    oacc = pp.tile([128, QT], dt.float32, tag="oacc", bufs=1)
                    pts = [None] * nch

                    def chunk_info(i):
                        if i < 4 * t:
                            return i, 0
                        j = i - 4 * t
                        return 4 * t + j, 128 * j

                    def emit_score(i):
                        c, qoff = chunk_info(i)
                        wd = QT - qoff
                        stp = pp.tile([128, QT], dt.float32, tag="stp")
                        nc.tensor.matmul(stp[:, 0:wd], kT[:, c, :],
                                         qT[:, h, 4 * t + qoff // 128:4 * t + 4, :],
                                         start=True, stop=True)
                        pt = wp.tile([128, QT], dt.bfloat16, tag="pt", bufs=4)
                        nc.scalar.activation(pt[:, 0:wd], stp[:, 0:wd], AF.Exp)
                        if i >= 4 * t:
                            nc.gpsimd.tensor_tensor(out=pt[:, 0:KC], in0=pt[:, 0:KC],
                                                    in1=tri_sb[:], op=AO.mult)
                        pts[i] = (pt, c, qoff, wd)

                    def emit_acc(i, last):
                        pt, c, qoff, wd = pts[i]
                        nc.tensor.matmul(lacc[:, qoff:QT], ones128[:], pt[:, 0:wd],
                                         start=(i == 0), stop=last)
                        nc.tensor.matmul(oacc[:, qoff:QT], v_sb[:, c, :], pt[:, 0:wd],
                                         start=(i == 0), stop=last)

                    emit_score(0)
                    for i in range(1, nch):
                        emit_score(i)
                        emit_acc(i - 1, last=False)
                    emit_acc(nch - 1, last=True)

                    # out = oacc / (lacc + exp(sink))
                    tmp = wp.tile([128, QT], dt.float32, tag="tmp")
                    nc.vector.tensor_scalar(out=tmp[:], in0=lacc[:],
                                            scalar1=es_sb[:, h:h + 1], scalar2=None,
                                            op0=AO.add)
                    rr = wp.tile([128, QT], dt.float32, tag="rr")
                    nc.vector.reciprocal_approx_fast(rr[:], tmp[:])
                    att = wp.tile([128, QT], dt.bfloat16, tag="att")
                    nc.vector.tensor_tensor(out=att[:], in0=oacc[:], in1=rr[:], op=AO.mult)
                    nc.scalar.dma_start(
                        ag_ins[t][:].rearrange("(h p) q -> p h q", p=128)[:, h, :], att[:])
                nc.gpsimd.collective_compute(
                    "AllGather", AO.bypass,
                    replica_groups=[list(range(N_CORES))],
                    ins=[ag_ins[t][:].opt()], outs=[ag_outs[t][:].opt()],
                )

            def wo_part(t):
                agr = ag_outs[t][:].rearrange("(c p) q -> p c q", p=128)
                aT = wp.tile([128, NC, QT], dt.bfloat16, tag="aT", bufs=2)
                nc.sync.dma_start(aT[:, 0:8, :], agr[:, 0:8, :])
                nc.sync.dma_start(aT[:, 8:16, :], agr[:, 8:16, :])
                for tt in range(QT // 128):
                    qsl = slice(tt * 128, (tt + 1) * 128)
                    yp = pp.tile([128, 512], dt.float32, tag="mm")
                    for c in range(NC):
                        nc.tensor.matmul(yp[:, 0:HL * DH], aT[:, c, qsl], wot_sb[:, c, :],
                                         start=(c == 0), stop=(c == NC - 1))
                    ysb = wp.tile([128, HL * DH], dt.float32, tag="ysb")
                    nc.vector.tensor_copy(ysb[:], yp[:, 0:HL * DH])
                    nc.sync.dma_start(y_out[t * QT + tt * 128:t * QT + (tt + 1) * 128, :],
                                      ysb[:])

            # ---- emission: all local work first, wo (collective-dependent) last ----
            for st in range(NST):
                proj(st)
                if st >= 4 and st % 4 == 0:
                    attn_group(st // 4 - 1)
            attn_group(NQT - 1)
            # pin the collective-dependent wo parts after ALL local work in
            # every engine queue (the Tile scheduler otherwise hoists them
            # ahead of later attention groups, stalling the PE FIFO on the
            # AllGather)
            for t in range(NQT):
                with tc.tile_wait_until(1.0 + 0.1 * t):
                    wo_part(t)

    nc.compile()
    return nc


def prep_inputs(x, freqs_cis, wq, wk, wv, wo, sinks):
    """Host-side sharding/layout prep. Returns in_maps for the 8 cores.

    All tensors are pre-tiled partition-major ([p, ...]) so DMAs move
    long contiguous per-partition runs.
    """
    x2 = np.ascontiguousarray(np.asarray(x, np.float32).reshape(S, D))
    xt = x2.T.astype(BF16)                                    # [D, S] = [(c p), (st s)]
    xt_h = np.ascontiguousarray(
        xt.reshape(NC, 128, NST, 128).transpose(1, 2, 0, 3).reshape(128, NST * NC * 128))

    fc = np.asarray(freqs_cis, np.float32)
    cos, sin = fc[:, :, 0], fc[:, :, 1]
    c1 = np.repeat(cos, 2, axis=1)             # [S, 128] pair-interleaved
    s1 = np.repeat(sin, 2, axis=1)
    cbar = np.concatenate([c1, c1, c1], axis=1).astype(np.float32)   # [S, 384] q0|q1|k
    sbar = np.concatenate([s1, s1, s1], axis=1).astype(np.float32)
    sbar[:, 0::2] *= -1.0                      # even outputs get -sin
    cbar_h = np.ascontiguousarray(
        cbar.reshape(NST, 128, EW).transpose(1, 0, 2).reshape(128, NST * EW)).astype(BF16)
    sbar_h = np.ascontiguousarray(
        sbar.reshape(NST, 128, EW).transpose(1, 0, 2).reshape(128, NST * EW)).astype(BF16)

    kr = np.arange(KC)[:, None]
    qr = np.arange(KC)[None, :]
    trimask = (qr >= kr).astype(np.float32).astype(BF16)      # [128, 128]

    wq = np.asarray(wq, np.float32)
    wk = np.asarray(wk, np.float32)
    wv = np.asarray(wv, np.float32)
    wo = np.asarray(wo, np.float32)
    sinks = np.asarray(sinks, np.float32)

    in_maps = []
    for d in range(N_CORES):
        kv = d // 2
        es = np.exp(sinks[2 * d:2 * d + 2]).astype(np.float32)
        wqkv = np.concatenate([
            wq[d * 256:(d + 1) * 256, :].T,
            wk[kv * 128:(kv + 1) * 128, :].T,
            wv[kv * 128:(kv + 1) * 128, :].T,
        ], axis=1).astype(BF16)                               # [D, 512] = [(c p), e]
        wqkv_h = np.ascontiguousarray(
            wqkv.reshape(NC, 128, 512).transpose(1, 0, 2).reshape(128, NC * 512))
        wotd = np.ascontiguousarray(wo[d * 256:(d + 1) * 256, :].T).astype(BF16)
        wot_h = np.ascontiguousarray(
            wotd.reshape(NC, 128, HL * DH).transpose(1, 0, 2).reshape(128, NC * HL * DH))
        in_maps.append({
            "xt": xt_h,
            "wqkv": wqkv_h,
            "wot": wot_h,
            "cbar": cbar_h,
            "sbar": sbar_h,
            "trimask": trimask,
            "es": np.repeat(es[None, :], 128, axis=0).astype(np.float32),
        })
    return in_maps


_CACHED = {}


def kernel(x, freqs_cis, wq, wk, wv, wo, sinks):
    if "nc" not in _CACHED:
        _CACHED["nc"] = build()
    nc = _CACHED["nc"]
    in_maps = prep_inputs(x, freqs_cis, wq, wk, wv, wo, sinks)
    res = run_bass_kernel_spmd(nc, in_maps, list(range(N_CORES)), trace=False)
    y = np.concatenate([res.results[d]["y"] for d in range(N_CORES)], axis=1)
    return y.reshape(1, S, D).astype(np.float32)


# revision 9
# speedup vs baseline: 1.1317x; 1.0141x over previous
"""Distributed Bass kernel for nn_Attention_32701880992127 on 8 TRN2 NeuronCores.

Sharding (tensor parallel over heads): core d owns q-heads {2d, 2d+1} and
kv-head d//2 (GQA consecutive-repeat mapping). wq/wk/wv are column-split,
wo is split along its OUTPUT dim so each core produces a distinct column
slice of the final output from the AllGathered attention features.

All matmuls run in bf16 (f32 PSUM accumulation); elementwise math stays f32.
Softmax needs no max-subtraction (qk-norm bounds the scores); the sink
correction folds into the denominator:
    out_h = (sum_k exp(s_qk) v_k) / (exp(sink_h) + sum_k exp(s_qk)).
Scores are computed transposed (ST[k, q]) so exp's output directly feeds the
PV matmul as the moving operand. The causal diagonal 512-block is processed
as 4 k-chunks with shrinking q-windows; only the 128x128 diagonal block of
each chunk needs a (shared triangular) mask, applied on the GpSimd engine.

Engine balance: qT/kT transposes run on the PE (XBAR DMA transposes cost
~1.2us each of serial queue time); qk-norm is one fused square+accum DVE op per
head plus reciprocal+Sqrt; rope for q0|q1|k runs merged in 4 wide DVE ops
(pair-swap via stride-2 APs, sin sign pre-folded into the host table).
All DRAM inputs are pre-tiled partition-major so every DMA moves >=2KB
contiguous runs (the DMA queues are packet-rate limited). Emission keeps the
PE FIFO free of collective-dependent work until all local attention is done.
"""
import numpy as np
import ml_dtypes

import concourse.mybir as mybir
import concourse.tile as tile
from concourse import bacc
from concourse.bass_utils import run_bass_kernel_spmd
from concourse.masks import make_identity

dt = mybir.dt
AO = mybir.AluOpType
AF = mybir.ActivationFunctionType
BF16 = ml_dtypes.bfloat16

N_CORES = 8
S = 2048            # sequence length
D = 2048            # model dim
DH = 128            # head dim
HL = 2              # local q heads per core
NC = 16             # d-chunks of 128
NST = 16            # s-tiles of 128
QT = 512            # attention q tile
NQT = S // QT
KC = 128            # attention k chunk
EW = 384            # rope width: q0|q1|k
RMS_EPS = 1.1920929e-07


def build():
    nc = bacc.Bacc("TRN2", target_bir_lowering=False, debug=False, num_devices=N_CORES)

    # all inputs pre-tiled partition-major on the host: [p, ...] with long
    # contiguous per-partition runs
    xt = nc.dram_tensor("xt", [128, NST * NC * 128], dt.bfloat16,
                        kind="ExternalInput").ap()            # [p, st, c, s]
    wqkv = nc.dram_tensor("wqkv", [128, NC * 512], dt.bfloat16,
                          kind="ExternalInput").ap()          # [p, c, e]
    wot = nc.dram_tensor("wot", [128, NC * HL * DH], dt.bfloat16,
                         kind="ExternalInput").ap()           # [p, c, e]
    cbar = nc.dram_tensor("cbar", [128, NST * EW], dt.bfloat16,
                          kind="ExternalInput").ap()          # [p, st, e] cos for q0|q1|k
    sbar = nc.dram_tensor("sbar", [128, NST * EW], dt.bfloat16,
                          kind="ExternalInput").ap()          # sign-folded sin
    trimask = nc.dram_tensor("trimask", [KC, KC], dt.bfloat16, kind="ExternalInput").ap()
    esd = nc.dram_tensor("es", [128, HL], dt.float32, kind="ExternalInput").ap()
    y_out = nc.dram_tensor("y", [S, HL * DH], dt.float32, kind="ExternalOutput").ap()

    with tile.TileContext(nc) as tc:
        with (
            tc.tile_pool(name="const", bufs=1) as cp,
            tc.tile_pool(name="work", bufs=2) as wp,
            tc.tile_pool(name="psum", bufs=2, space="PSUM") as pp,
            tc.tile_pool(name="dram", bufs=1, space="DRAM") as dp,
        ):
            # ---- persistent tiles ----
            wqkv_sb = cp.tile([128, NC, 512], dt.bfloat16, tag="wqkv")
            xt_sb = cp.tile([128, NST, NC, 128], dt.bfloat16, tag="xt")
            wot_sb = cp.tile([128, NC, HL * DH], dt.bfloat16, tag="wot")
            cbar_sb = cp.tile([128, NST, EW], dt.bfloat16, tag="cbar")
            sbar_sb = cp.tile([128, NST, EW], dt.bfloat16, tag="sbar")
            tri_sb = cp.tile([128, KC], dt.bfloat16, tag="tri")
            es_sb = cp.tile([128, HL], dt.float32, tag="es")
            ones128 = cp.tile([128, 128], dt.bfloat16, tag="ones128")
            nc.vector.memset(ones128[:], 1.0)
            ident = cp.tile([128, 128], dt.bfloat16, tag="ident")
            make_identity(nc, ident[:])

            qT = cp.tile([128, HL, NST, 128], dt.bfloat16, tag="qT")  # [dh, h, st, s]
            kT = cp.tile([128, NST, 128], dt.bfloat16, tag="kT")      # [dh, st, s]
            v_sb = cp.tile([128, NST, DH], dt.bfloat16, tag="v")      # [s, st, dh]

            # ---- AllGather bounce buffers (one pair per q-group) ----
            ag_ins = [dp.tile([HL * 128, QT], dt.bfloat16, name=f"ag_in{i}")
                      for i in range(NQT)]
            ag_outs = [dp.tile([N_CORES * HL * 128, QT], dt.bfloat16, addr_space="Shared",
                               name=f"ag_out{i}") for i in range(NQT)]

            # ---- input DMA schedule ----
            # sync stays short early (it also runs the qT/kT XBAR transposes);
            # scalar carries most inputs, interleaved so each piece lands just
            # before its consumer.
            xts = xt.rearrange("p (st e) -> p st e", st=NST)
            cbr = cbar.rearrange("p (st e) -> p st e", st=NST)
            sbr = sbar.rearrange("p (st e) -> p st e", st=NST)
            wqr = wqkv.rearrange("p (c e) -> p c e", c=NC)

            xts4 = xt.rearrange("p (st c e) -> p st c e", st=NST, c=NC)
            nc.scalar.dma_start(xt_sb[:, 0, 0:4, :], xts4[:, 0, 0:4, :])
            nc.scalar.dma_start(xt_sb[:, 0, 4:16, :], xts4[:, 0, 4:16, :])
            for g in range(4):
                nc.sync.dma_start(wqkv_sb[:, 4 * g:4 * g + 4, :], wqr[:, 4 * g:4 * g + 4, :])
            nc.scalar.dma_start(xt_sb[:, 1, :, :], xts[:, 1, :])
            nc.scalar.dma_start(cbar_sb[:, 0:4, :], cbr[:, 0:4, :])
            nc.scalar.dma_start(sbar_sb[:, 0:4, :], sbr[:, 0:4, :])
            nc.scalar.dma_start(es_sb[:], esd)
            nc.sync.dma_start(xt_sb[:, 2, :, :], xts[:, 2, :])
            nc.sync.dma_start(xt_sb[:, 3, :, :], xts[:, 3, :])
            nc.sync.dma_start(tri_sb[:], trimask)
            for g in range(1, 4):
                nc.scalar.dma_start(cbar_sb[:, 4 * g:4 * g + 4, :], cbr[:, 4 * g:4 * g + 4, :])
                nc.scalar.dma_start(sbar_sb[:, 4 * g:4 * g + 4, :], sbr[:, 4 * g:4 * g + 4, :])
            for st in range(4, NST):
                eng = nc.sync if st in (4, 6) else nc.scalar
                eng.dma_start(xt_sb[:, st, :, :], xts[:, st, :])
            nc.scalar.dma_start(wot_sb[:], wot.rearrange("p (c e) -> p c e", c=NC))

            def proj(st):
                mm = pp.tile([128, 512], dt.float32, tag="mm")  # q[0:256] | k[256:384] | v[384:512]
                for c in range(NC):
                    nc.tensor.matmul(mm[:], xt_sb[:, st, c, :], wqkv_sb[:, c, :],
                                     start=(c == 0), stop=(c == NC - 1))

                # evacuate PSUM: q|k to f32 SBUF, v to bf16
                qk = wp.tile([128, EW], dt.float32, tag="qk", bufs=4)
                nc.vector.tensor_copy(qk[:], mm[:, 0:EW])
                nc.vector.tensor_copy(v_sb[:, st, :], mm[:, EW:512])

                # qk-norm: ssq via fused square+accum on DVE
                ssq = wp.tile([128, 4], dt.float32, tag="ssq")
                scr = wp.tile([128, 128], dt.float32, tag="scr")
                for i in range(3):
                    nc.vector.scalar_tensor_tensor(
                        out=scr[:], in0=qk[:, i * DH:(i + 1) * DH], scalar=1.0,
                        in1=qk[:, i * DH:(i + 1) * DH], op0=AO.bypass, op1=AO.mult,
                        accum_out=ssq[:, i:i + 1])
                # q: rsqrt(mean+eps) = sqrt(1/(ssq/128+eps)); k: extra 1/sqrt(128)
                nc.vector.tensor_scalar(out=ssq[:, 0:2], in0=ssq[:, 0:2],
                                        scalar1=1.0 / DH, scalar2=RMS_EPS,
                                        op0=AO.mult, op1=AO.add)
                nc.vector.tensor_scalar(out=ssq[:, 2:3], in0=ssq[:, 2:3],
                                        scalar1=DH * RMS_EPS, scalar2=None, op0=AO.add)
                rcp = wp.tile([128, 4], dt.float32, tag="rcp")
                nc.vector.reciprocal_approx_fast(rcp[:, 0:3], ssq[:, 0:3])
                rs = wp.tile([128, 4], dt.float32, tag="rs")
                nc.scalar.activation(rs[:, 0:3], rcp[:, 0:3], AF.Sqrt)

                # merged rope for q0|q1|k: u = qk*cos; w = pairswap(qk)*(+-sin)
                u1 = wp.tile([128, EW], dt.float32, tag="u1")
                w = wp.tile([128, EW], dt.float32, tag="w")
                nc.vector.tensor_tensor(out=u1[:], in0=qk[:], in1=cbar_sb[:, st, :],
                                        op=AO.mult)
                nc.vector.tensor_tensor(out=w[:, 0:EW:2], in0=qk[:, 1:EW:2],
                                        in1=sbar_sb[:, st, 0:EW:2], op=AO.mult)
                nc.vector.tensor_tensor(out=w[:, 1:EW:2], in0=qk[:, 0:EW:2],
                                        in1=sbar_sb[:, st, 1:EW:2], op=AO.mult)
                nc.vector.tensor_add(out=u1[:], in0=u1[:], in1=w[:])
                qhat = wp.tile([128, HL * DH], dt.bfloat16, tag="qhat")
                khat = wp.tile([128, DH], dt.bfloat16, tag="khat")
                for h in range(HL):
                    nc.vector.tensor_scalar(out=qhat[:, h * DH:(h + 1) * DH],
                                            in0=u1[:, h * DH:(h + 1) * DH],
                                            scalar1=rs[:, h:h + 1], scalar2=None, op0=AO.mult)
                nc.vector.tensor_scalar(out=khat[:], in0=u1[:, 2 * DH:EW],
                                        scalar1=rs[:, 2:3], scalar2=None, op0=AO.mult)

                # PE transposes -> qT / kT (PSUM copies on ACT)
                for h in range(HL):
                    tp = pp.tile([128, 128], dt.bfloat16, tag="tp")
                    nc.tensor.transpose(tp[:], qhat[:, h * DH:(h + 1) * DH], ident[:])
                    nc.scalar.copy(qT[:, h, st, :], tp[:])
                tpk = pp.tile([128, 128], dt.bfloat16, tag="tp")
                nc.tensor.transpose(tpk[:], khat[:], ident[:])
                nc.scalar.copy(kT[:, st, :], tpk[:])

            def attn_group(t):
                # chunk i: i < 4t -> full k-chunk c=i over q cols [0:512)
                #          i >= 4t -> diagonal chunk c=4t+j over q cols [128j:512)
                nch = 4 * t + 4
                for h in range(HL):
                    lacc = pp.tile([128, QT], dt.float32, tag="lacc", bufs=1)
                    oacc = pp.tile([128, QT], dt.float32, tag="oacc", bufs=1)
                    pts = [None] * nch

                    def chunk_info(i):
                        if i < 4 * t:
                            return i, 0
                        j = i - 4 * t
                        return 4 * t + j, 128 * j

                    def emit_score(i):
                        c, qoff = chunk_info(i)
                        wd = QT - qoff
                        stp = pp.tile([128, QT], dt.float32, tag="stp")
                        nc.tensor.matmul(stp[:, 0:wd], kT[:, c, :],
                                         qT[:, h, 4 * t + qoff // 128:4 * t + 4, :],
                                         start=True, stop=True)
                        pt = wp.tile([128, QT], dt.bfloat16, tag="pt", bufs=4)
                        nc.scalar.activation(pt[:, 0:wd], stp[:, 0:wd], AF.Exp)
                        if i >= 4 * t:
                            nc.gpsimd.tensor_tensor(out=pt[:, 0:KC], in0=pt[:, 0:KC],
                                                    in1=tri_sb[:], op=AO.mult)
                        pts[i] = (pt, c, qoff, wd)

                    def emit_acc(i, last):
                        pt, c, qoff, wd = pts[i]
                        nc.tensor.matmul(lacc[:, qoff:QT], ones128[:], pt[:, 0:wd],
                                         start=(i == 0), stop=last)
                        nc.tensor.matmul(oacc[:, qoff:QT], v_sb[:, c, :], pt[:, 0:wd],
                                         start=(i == 0), stop=last)

                    emit_score(0)
                    for i in range(1, nch):
                        emit_score(i)
                        emit_acc(i - 1, last=False)
                    emit_acc(nch - 1, last=True)

                    # out = oacc / (lacc + exp(sink))
                    tmp = wp.tile([128, QT], dt.float32, tag="tmp")
                    nc.vector.tensor_scalar(out=tmp[:], in0=lacc[:],
                                            scalar1=es_sb[:, h:h + 1], scalar2=None,
                                            op0=AO.add)
                    rr = wp.tile([128, QT], dt.float32, tag="rr")
                    nc.vector.reciprocal_approx_fast(rr[:], tmp[:])
                    att = wp.tile([128, QT], dt.bfloat16, tag="att")
                    nc.vector.tensor_tensor(out=att[:], in0=oacc[:], in1=rr[:], op=AO.mult)
                    nc.scalar.dma_start(
                        ag_ins[t][:].rearrange("(h p) q -> p h q", p=128)[:, h, :], att[:])
                nc.gpsimd.collective_compute(
                    "AllGather", AO.bypass,
                    replica_groups=[list(range(N_CORES))],
                    ins=[ag_ins[t][:].opt()], outs=[ag_outs[t][:].opt()],
                )

            def wo_part(t):
                agr = ag_outs[t][:].rearrange("(c p) q -> p c q", p=128)
                aT = wp.tile([128, NC, QT], dt.bfloat16, tag="aT", bufs=2)
                nc.sync.dma_start(aT[:, 0:8, :], agr[:, 0:8, :])
                nc.sync.dma_start(aT[:, 8:16, :], agr[:, 8:16, :])
                for tt in range(QT // 128):
                    qsl = slice(tt * 128, (tt + 1) * 128)
                    yp = pp.tile([128, 512], dt.float32, tag="mm")
                    for c in range(NC):
                        nc.tensor.matmul(yp[:, 0:HL * DH], aT[:, c, qsl], wot_sb[:, c, :],
                                         start=(c == 0), stop=(c == NC - 1))
                    ysb = wp.tile([128, HL * DH], dt.float32, tag="ysb")
                    nc.vector.tensor_copy(ysb[:], yp[:, 0:HL * DH])
                    nc.scalar.dma_start(y_out[t * QT + tt * 128:t * QT + (tt + 1) * 128, :],
                                        ysb[:])

            # ---- emission: all local work first, wo (collective-dependent) last ----
            for st in range(NST):
                proj(st)
                if st >= 4 and st % 4 == 0:
                    attn_group(st // 4 - 1)
            attn_group(NQT - 1)
            # pin the collective-dependent wo parts after ALL local work in
            # every engine queue (the Tile scheduler otherwise hoists them
            # ahead of later attention groups, stalling the PE FIFO on the
            # AllGather)
            for t in range(NQT):
                with tc.tile_wait_until(1.0 + 0.1 * t):
                    wo_part(t)

    nc.compile()
    return nc


def prep_inputs(x, freqs_cis, wq, wk, wv, wo, sinks):
    """Host-side sharding/layout prep. Returns in_maps for the 8 cores.

    All tensors are pre-tiled partition-major ([p, ...]) so DMAs move
    long contiguous per-partition runs.
    """
    x2 = np.ascontiguousarray(np.asarray(x, np.float32).reshape(S, D))
    xt = x2.T.astype(BF16)                                    # [D, S] = [(c p), (st s)]
    xt_h = np.ascontiguousarray(
        xt.reshape(NC, 128, NST, 128).transpose(1, 2, 0, 3).reshape(128, NST * NC * 128))

    fc = np.asarray(freqs_cis, np.float32)
    cos, sin = fc[:, :, 0], fc[:, :, 1]
    c1 = np.repeat(cos, 2, axis=1)             # [S, 128] pair-interleaved
    s1 = np.repeat(sin, 2, axis=1)
    cbar = np.concatenate([c1, c1, c1], axis=1).astype(np.float32)   # [S, 384] q0|q1|k
    sbar = np.concatenate([s1, s1, s1], axis=1).astype(np.float32)
    sbar[:, 0::2] *= -1.0                      # even outputs get -sin
    cbar_h = np.ascontiguousarray(
        cbar.reshape(NST, 128, EW).transpose(1, 0, 2).reshape(128, NST * EW)).astype(BF16)
    sbar_h = np.ascontiguousarray(
        sbar.reshape(NST, 128, EW).transpose(1, 0, 2).reshape(128, NST * EW)).astype(BF16)

    kr = np.arange(KC)[:, None]
    qr = np.arange(KC)[None, :]
    trimask = (qr >= kr).astype(np.float32).astype(BF16)      # [128, 128]

    wq = np.asarray(wq, np.float32)
    wk = np.asarray(wk, np.float32)
    wv = np.asarray(wv, np.float32)
    wo = np.asarray(wo, np.float32)
    sinks = np.asarray(sinks, np.float32)

    in_maps = []
    for d in range(N_CORES):
        kv = d // 2
        es = np.exp(sinks[2 * d:2 * d + 2]).astype(np.float32)
        wqkv = np.concatenate([
            wq[d * 256:(d + 1) * 256, :].T,
            wk[kv * 128:(kv + 1) * 128, :].T,
            wv[kv * 128:(kv + 1) * 128, :].T,
        ], axis=1).astype(BF16)                               # [D, 512] = [(c p), e]
        wqkv_h = np.ascontiguousarray(
            wqkv.reshape(NC, 128, 512).transpose(1, 0, 2).reshape(128, NC * 512))
        wotd = np.ascontiguousarray(wo[d * 256:(d + 1) * 256, :].T).astype(BF16)
        wot_h = np.ascontiguousarray(
            wotd.reshape(NC, 128, HL * DH).transpose(1, 0, 2).reshape(128, NC * HL * DH))
        in_maps.append({
            "xt": xt_h,
            "wqkv": wqkv_h,
            "wot": wot_h,
            "cbar": cbar_h,
            "sbar": sbar_h,
            "trimask": trimask,
            "es": np.repeat(es[None, :], 128, axis=0).astype(np.float32),
        })
    return in_maps


_CACHED = {}


def kernel(x, freqs_cis, wq, wk, wv, wo, sinks):
    if "nc" not in _CACHED:
        _CACHED["nc"] = build()
    nc = _CACHED["nc"]
    in_maps = prep_inputs(x, freqs_cis, wq, wk, wv, wo, sinks)
    res = run_bass_kernel_spmd(nc, in_maps, list(range(N_CORES)), trace=False)
    y = np.concatenate([res.results[d]["y"] for d in range(N_CORES)], axis=1)
    return y.reshape(1, S, D).astype(np.float32)


# revision 12
# speedup vs baseline: 1.2008x; 1.0611x over previous
"""Distributed Bass kernel for nn_Attention_32701880992127 on 8 TRN2 NeuronCores.

Sharding (tensor parallel over heads): core d owns q-heads {2d, 2d+1} and
kv-head d//2 (GQA consecutive-repeat mapping). wq/wk/wv are column-split,
wo is split along its OUTPUT dim so each core produces a distinct column
slice of the final output from the AllGathered attention features.

All matmuls run in bf16 (f32 PSUM accumulation); elementwise math stays f32.
Softmax needs no max-subtraction (qk-norm bounds the scores); the sink
correction folds into the denominator:
    out_h = (sum_k exp(s_qk) v_k) / (exp(sink_h) + sum_k exp(s_qk)).
Scores are computed transposed (ST[k, q]) so exp's output directly feeds the
PV matmul as the moving operand. The causal diagonal 512-block is processed
as 4 k-chunks with shrinking q-windows; only the 128x128 diagonal block of
each chunk needs a (shared triangular) mask, applied on the GpSimd engine.

Engine balance: qT/kT transposes run on the PE (XBAR DMA transposes cost
~1.2us each of serial queue time); qk-norm is one fused square+accum DVE op per
head plus reciprocal+Sqrt; rope for q0|q1|k runs merged in 4 wide DVE ops
(pair-swap via stride-2 APs, sin sign pre-folded into the host table).
All DRAM inputs are pre-tiled partition-major so every DMA moves >=2KB
contiguous runs (the DMA queues are packet-rate limited). Emission keeps the
PE FIFO free of collective-dependent work until all local attention is done.
"""
import numpy as np
import ml_dtypes

import concourse.mybir as mybir
import concourse.tile as tile
from concourse import bacc
from concourse.bass_utils import run_bass_kernel_spmd
from concourse.masks import make_identity

dt = mybir.dt
AO = mybir.AluOpType
AF = mybir.ActivationFunctionType
BF16 = ml_dtypes.bfloat16

N_CORES = 8
S = 2048            # sequence length
D = 2048            # model dim
DH = 128            # head dim
HL = 2              # local q heads per core
NC = 16             # d-chunks of 128
NST = 16            # s-tiles of 128
QT = 512            # attention q tile
NQT = S // QT
KC = 128            # attention k chunk
EW = 384            # rope width: q0|q1|k
RMS_EPS = 1.1920929e-07


def build():
    nc = bacc.Bacc("TRN2", target_bir_lowering=False, debug=False, num_devices=N_CORES)

    # all inputs pre-tiled partition-major on the host: [p, ...] with long
    # contiguous per-partition runs
    xt = nc.dram_tensor("xt", [128, NST * NC * 128], dt.bfloat16,
                        kind="ExternalInput").ap()            # [p, st, c, s]
    wqkv = nc.dram_tensor("wqkv", [128, NC * 512], dt.bfloat16,
                          kind="ExternalInput").ap()          # [p, c, e]
    wot = nc.dram_tensor("wot", [128, NC * HL * DH], dt.bfloat16,
                         kind="ExternalInput").ap()           # [p, c, e]
    cbar = nc.dram_tensor("cbar", [128, NST * EW], dt.bfloat16,
                          kind="ExternalInput").ap()          # [p, st, e] cos for q0|q1|k
    sbar = nc.dram_tensor("sbar", [128, NST * EW], dt.bfloat16,
                          kind="ExternalInput").ap()          # sign-folded sin
    trimask = nc.dram_tensor("trimask", [KC, KC], dt.bfloat16, kind="ExternalInput").ap()
    esd = nc.dram_tensor("es", [128, HL], dt.float32, kind="ExternalInput").ap()
    y_out = nc.dram_tensor("y", [S, HL * DH], dt.float32, kind="ExternalOutput").ap()

    with tile.TileContext(nc) as tc:
        with (
            tc.tile_pool(name="const", bufs=1) as cp,
            tc.tile_pool(name="work", bufs=2) as wp,
            tc.tile_pool(name="psum", bufs=2, space="PSUM") as pp,
            tc.tile_pool(name="dram", bufs=1, space="DRAM") as dp,
        ):
            # ---- persistent tiles ----
            wqkv_sb = cp.tile([128, NC, 512], dt.bfloat16, tag="wqkv")
            xt_sb = cp.tile([128, NST, NC, 128], dt.bfloat16, tag="xt")
            wot_sb = cp.tile([128, NC, HL * DH], dt.bfloat16, tag="wot")
            cbar_sb = cp.tile([128, NST, EW], dt.bfloat16, tag="cbar")
            sbar_sb = cp.tile([128, NST, EW], dt.bfloat16, tag="sbar")
            tri_sb = cp.tile([128, KC], dt.bfloat16, tag="tri")
            es_sb = cp.tile([128, HL], dt.float32, tag="es")
            ones128 = cp.tile([128, 128], dt.bfloat16, tag="ones128")
            nc.vector.memset(ones128[:], 1.0)
            ident = cp.tile([128, 128], dt.bfloat16, tag="ident")
            make_identity(nc, ident[:])

            qT = cp.tile([128, HL, NST, 128], dt.bfloat16, tag="qT")  # [dh, h, st, s]
            kT = cp.tile([128, NST, 128], dt.bfloat16, tag="kT")      # [dh, st, s]
            v_sb = cp.tile([128, NST, DH], dt.bfloat16, tag="v")      # [s, st, dh]

            # ---- AllGather bounce buffers (one pair per q-group) ----
            ag_ins = [dp.tile([HL * 128, QT], dt.bfloat16, name=f"ag_in{i}")
                      for i in range(NQT)]
            ag_outs = [dp.tile([N_CORES * HL * 128, QT], dt.bfloat16, addr_space="Shared",
                               name=f"ag_out{i}") for i in range(NQT)]

            # ---- input DMA schedule ----
            # sync stays short early (it also runs the qT/kT XBAR transposes);
            # scalar carries most inputs, interleaved so each piece lands just
            # before its consumer.
            xts = xt.rearrange("p (st e) -> p st e", st=NST)
            cbr = cbar.rearrange("p (st e) -> p st e", st=NST)
            sbr = sbar.rearrange("p (st e) -> p st e", st=NST)
            wqr = wqkv.rearrange("p (c e) -> p c e", c=NC)

            xts4 = xt.rearrange("p (st c e) -> p st c e", st=NST, c=NC)
            nc.scalar.dma_start(xt_sb[:, 0, 0:4, :], xts4[:, 0, 0:4, :])
            nc.scalar.dma_start(xt_sb[:, 0, 4:16, :], xts4[:, 0, 4:16, :])
            for g in range(4):
                nc.sync.dma_start(wqkv_sb[:, 4 * g:4 * g + 4, :], wqr[:, 4 * g:4 * g + 4, :])
            nc.scalar.dma_start(xt_sb[:, 1, :, :], xts[:, 1, :])
            nc.scalar.dma_start(cbar_sb[:, 0:4, :], cbr[:, 0:4, :])
            nc.scalar.dma_start(sbar_sb[:, 0:4, :], sbr[:, 0:4, :])
            nc.scalar.dma_start(es_sb[:], esd)
            nc.sync.dma_start(xt_sb[:, 2, :, :], xts[:, 2, :])
            nc.sync.dma_start(xt_sb[:, 3, :, :], xts[:, 3, :])
            nc.sync.dma_start(tri_sb[:], trimask)
            for g in range(1, 4):
                nc.scalar.dma_start(cbar_sb[:, 4 * g:4 * g + 4, :], cbr[:, 4 * g:4 * g + 4, :])
                nc.scalar.dma_start(sbar_sb[:, 4 * g:4 * g + 4, :], sbr[:, 4 * g:4 * g + 4, :])
            for st in range(4, NST):
                eng = nc.sync if st in (4, 6) else nc.scalar
                eng.dma_start(xt_sb[:, st, :, :], xts[:, st, :])
            nc.scalar.dma_start(wot_sb[:], wot.rearrange("p (c e) -> p c e", c=NC))

            def proj(st):
                mm = pp.tile([128, 512], dt.float32, tag="mm")  # q[0:256] | k[256:384] | v[384:512]
                for c in range(NC):
                    nc.tensor.matmul(mm[:], xt_sb[:, st, c, :], wqkv_sb[:, c, :],
                                     start=(c == 0), stop=(c == NC - 1))

                # evacuate PSUM: q|k to f32 SBUF, v to bf16
                qk = wp.tile([128, EW], dt.float32, tag="qk", bufs=4)
                nc.vector.tensor_copy(qk[:], mm[:, 0:EW])
                nc.vector.tensor_copy(v_sb[:, st, :], mm[:, EW:512])

                # qk-norm: ssq via fused square+accum on DVE
                ssq = wp.tile([128, 4], dt.float32, tag="ssq")
                scr = wp.tile([128, 128], dt.float32, tag="scr")
                for i in range(3):
                    nc.vector.scalar_tensor_tensor(
                        out=scr[:], in0=qk[:, i * DH:(i + 1) * DH], scalar=1.0,
                        in1=qk[:, i * DH:(i + 1) * DH], op0=AO.bypass, op1=AO.mult,
                        accum_out=ssq[:, i:i + 1])
                # q: rsqrt(mean+eps) = sqrt(1/(ssq/128+eps)); k: extra 1/sqrt(128)
                nc.vector.tensor_scalar(out=ssq[:, 0:2], in0=ssq[:, 0:2],
                                        scalar1=1.0 / DH, scalar2=RMS_EPS,
                                        op0=AO.mult, op1=AO.add)
                nc.vector.tensor_scalar(out=ssq[:, 2:3], in0=ssq[:, 2:3],
                                        scalar1=DH * RMS_EPS, scalar2=None, op0=AO.add)
                rcp = wp.tile([128, 4], dt.float32, tag="rcp")
                nc.vector.reciprocal_approx_fast(rcp[:, 0:3], ssq[:, 0:3])
                rs = wp.tile([128, 4], dt.float32, tag="rs")
                nc.scalar.activation(rs[:, 0:3], rcp[:, 0:3], AF.Sqrt)

                # merged rope for q0|q1|k: u = qk*cos; w = pairswap(qk)*(+-sin)
                u1 = wp.tile([128, EW], dt.float32, tag="u1")
                w = wp.tile([128, EW], dt.float32, tag="w")
                nc.vector.tensor_tensor(out=u1[:], in0=qk[:], in1=cbar_sb[:, st, :],
                                        op=AO.mult)
                nc.vector.tensor_tensor(out=w[:, 0:EW:2], in0=qk[:, 1:EW:2],
                                        in1=sbar_sb[:, st, 0:EW:2], op=AO.mult)
                nc.vector.tensor_tensor(out=w[:, 1:EW:2], in0=qk[:, 0:EW:2],
                                        in1=sbar_sb[:, st, 1:EW:2], op=AO.mult)
                nc.vector.tensor_add(out=u1[:], in0=u1[:], in1=w[:])
                qhat = wp.tile([128, HL * DH], dt.bfloat16, tag="qhat")
                khat = wp.tile([128, DH], dt.bfloat16, tag="khat")
                for h in range(HL):
                    nc.vector.tensor_scalar(out=qhat[:, h * DH:(h + 1) * DH],
                                            in0=u1[:, h * DH:(h + 1) * DH],
                                            scalar1=rs[:, h:h + 1], scalar2=None, op0=AO.mult)
                nc.vector.tensor_scalar(out=khat[:], in0=u1[:, 2 * DH:EW],
                                        scalar1=rs[:, 2:3], scalar2=None, op0=AO.mult)

                # PE transposes -> qT / kT (PSUM copies on ACT)
                for h in range(HL):
                    tp = pp.tile([128, 128], dt.bfloat16, tag="tp")
                    nc.tensor.transpose(tp[:], qhat[:, h * DH:(h + 1) * DH], ident[:])
                    nc.scalar.copy(qT[:, h, st, :], tp[:])
                tpk = pp.tile([128, 128], dt.bfloat16, tag="tp")
                nc.tensor.transpose(tpk[:], khat[:], ident[:])
                nc.scalar.copy(kT[:, st, :], tpk[:])

            def attn_group(t):
                # chunk i: i < 4t -> full k-chunk c=i over q cols [0:512)
                #          i >= 4t -> diagonal chunk c=4t+j over q cols [128j:512)
                nch = 4 * t + 4

                def chunk_info(i):
                    if i < 4 * t:
                        return i, 0
                    j = i - 4 * t
                    return 4 * t + j, 128 * j

                # flat (h, i) task list with one-task software pipelining so
                # the exp of each chunk hides under the previous chunk's
                # accumulation matmuls, across head boundaries too
                accs = {}
                pts = {}

                def emit_score(h, i):
                    if i == 0:
                        lacc = pp.tile([128, QT], dt.float32, tag="lacc", bufs=1)
                        oacc = pp.tile([128, QT], dt.float32, tag="oacc", bufs=1)
                        accs[h] = (lacc, oacc)
                    c, qoff = chunk_info(i)
                    wd = QT - qoff
                    stp = pp.tile([128, QT], dt.float32, tag="stp")
                    nc.tensor.matmul(stp[:, 0:wd], kT[:, c, :],
                                     qT[:, h, 4 * t + qoff // 128:4 * t + 4, :],
                                     start=True, stop=True)
                    pt = wp.tile([128, QT], dt.bfloat16, tag="pt", bufs=4)
                    nc.scalar.activation(pt[:, 0:wd], stp[:, 0:wd], AF.Exp)
                    if i >= 4 * t:
                        nc.gpsimd.tensor_tensor(out=pt[:, 0:KC], in0=pt[:, 0:KC],
                                                in1=tri_sb[:], op=AO.mult)
                    pts[(h, i)] = (pt, c, qoff, wd)

                def emit_acc(h, i):
                    pt, c, qoff, wd = pts.pop((h, i))
                    lacc, oacc = accs[h]
                    last = i == nch - 1
                    nc.tensor.matmul(lacc[:, qoff:QT], ones128[:], pt[:, 0:wd],
                                     start=(i == 0), stop=last)
                    nc.tensor.matmul(oacc[:, qoff:QT], v_sb[:, c, :], pt[:, 0:wd],
                                     start=(i == 0), stop=last)
                    if last:
                        emit_finish(h)

                def emit_finish(h):
                    # out = oacc / (lacc + exp(sink))
                    lacc, oacc = accs[h]
                    tmp = wp.tile([128, QT], dt.float32, tag="tmp")
                    nc.vector.tensor_scalar(out=tmp[:], in0=lacc[:],
                                            scalar1=es_sb[:, h:h + 1], scalar2=None,
                                            op0=AO.add)
                    rr = wp.tile([128, QT], dt.float32, tag="rr")
                    nc.vector.reciprocal_approx_fast(rr[:], tmp[:])
                    att = wp.tile([128, QT], dt.bfloat16, tag="att")
                    nc.vector.tensor_tensor(out=att[:], in0=oacc[:], in1=rr[:], op=AO.mult)
                    nc.scalar.dma_start(
                        ag_ins[t][:].rearrange("(h p) q -> p h q", p=128)[:, h, :], att[:])

                tasks = [(h, i) for h in range(HL) for i in range(nch)]
                emit_score(*tasks[0])
                for j in range(1, len(tasks)):
                    emit_score(*tasks[j])
                    emit_acc(*tasks[j - 1])
                emit_acc(*tasks[-1])
                nc.gpsimd.collective_compute(
                    "AllGather", AO.bypass,
                    replica_groups=[list(range(N_CORES))],
                    ins=[ag_ins[t][:].opt()], outs=[ag_outs[t][:].opt()],
                )

            def wo_part(t):
                agr = ag_outs[t][:].rearrange("(c p) q -> p c q", p=128)
                aT = wp.tile([128, NC, QT], dt.bfloat16, tag="aT", bufs=2)
                nc.sync.dma_start(aT[:, 0:8, :], agr[:, 0:8, :])
                nc.sync.dma_start(aT[:, 8:16, :], agr[:, 8:16, :])
                for tt in range(QT // 128):
                    qsl = slice(tt * 128, (tt + 1) * 128)
                    yp = pp.tile([128, 512], dt.float32, tag="mm")
                    for c in range(NC):
                        nc.tensor.matmul(yp[:, 0:HL * DH], aT[:, c, qsl], wot_sb[:, c, :],
                                         start=(c == 0), stop=(c == NC - 1))
                    ysb = wp.tile([128, HL * DH], dt.float32, tag="ysb")
                    nc.vector.tensor_copy(ysb[:], yp[:, 0:HL * DH])
                    nc.scalar.dma_start(y_out[t * QT + tt * 128:t * QT + (tt + 1) * 128, :],
                                        ysb[:])

            # ---- emission: all local work first, wo (collective-dependent) last ----
            for st in range(NST):
                proj(st)
                if st >= 4 and st % 4 == 0:
                    attn_group(st // 4 - 1)
            attn_group(NQT - 1)
            # pin the collective-dependent wo parts after ALL local work in
            # every engine queue (the Tile scheduler otherwise hoists them
            # ahead of later attention groups, stalling the PE FIFO on the
            # AllGather)
            for t in range(NQT):
                with tc.tile_wait_until(1.0 + 0.1 * t):
                    wo_part(t)

    nc.compile()
    return nc


def prep_inputs(x, freqs_cis, wq, wk, wv, wo, sinks):
    """Host-side sharding/layout prep. Returns in_maps for the 8 cores.

    All tensors are pre-tiled partition-major ([p, ...]) so DMAs move
    long contiguous per-partition runs.
    """
    x2 = np.ascontiguousarray(np.asarray(x, np.float32).reshape(S, D))
    xt = x2.T.astype(BF16)                                    # [D, S] = [(c p), (st s)]
    xt_h = np.ascontiguousarray(
        xt.reshape(NC, 128, NST, 128).transpose(1, 2, 0, 3).reshape(128, NST * NC * 128))

    fc = np.asarray(freqs_cis, np.float32)
    cos, sin = fc[:, :, 0], fc[:, :, 1]
    c1 = np.repeat(cos, 2, axis=1)             # [S, 128] pair-interleaved
    s1 = np.repeat(sin, 2, axis=1)
    cbar = np.concatenate([c1, c1, c1], axis=1).astype(np.float32)   # [S, 384] q0|q1|k
    sbar = np.concatenate([s1, s1, s1], axis=1).astype(np.float32)
    sbar[:, 0::2] *= -1.0                      # even outputs get -sin
    cbar_h = np.ascontiguousarray(
        cbar.reshape(NST, 128, EW).transpose(1, 0, 2).reshape(128, NST * EW)).astype(BF16)
    sbar_h = np.ascontiguousarray(
        sbar.reshape(NST, 128, EW).transpose(1, 0, 2).reshape(128, NST * EW)).astype(BF16)

    kr = np.arange(KC)[:, None]
    qr = np.arange(KC)[None, :]
    trimask = (qr >= kr).astype(np.float32).astype(BF16)      # [128, 128]

    wq = np.asarray(wq, np.float32)
    wk = np.asarray(wk, np.float32)
    wv = np.asarray(wv, np.float32)
    wo = np.asarray(wo, np.float32)
    sinks = np.asarray(sinks, np.float32)

    in_maps = []
    for d in range(N_CORES):
        kv = d // 2
        es = np.exp(sinks[2 * d:2 * d + 2]).astype(np.float32)
        wqkv = np.concatenate([
            wq[d * 256:(d + 1) * 256, :].T,
            wk[kv * 128:(kv + 1) * 128, :].T,
            wv[kv * 128:(kv + 1) * 128, :].T,
        ], axis=1).astype(BF16)                               # [D, 512] = [(c p), e]
        wqkv_h = np.ascontiguousarray(
            wqkv.reshape(NC, 128, 512).transpose(1, 0, 2).reshape(128, NC * 512))
        wotd = np.ascontiguousarray(wo[d * 256:(d + 1) * 256, :].T).astype(BF16)
        wot_h = np.ascontiguousarray(
            wotd.reshape(NC, 128, HL * DH).transpose(1, 0, 2).reshape(128, NC * HL * DH))
        in_maps.append({
            "xt": xt_h,
            "wqkv": wqkv_h,
            "wot": wot_h,
            "cbar": cbar_h,
            "sbar": sbar_h,
            "trimask": trimask,
            "es": np.repeat(es[None, :], 128, axis=0).astype(np.float32),
        })
    return in_maps


_CACHED = {}


def kernel(x, freqs_cis, wq, wk, wv, wo, sinks):
    if "nc" not in _CACHED:
        _CACHED["nc"] = build()
    nc = _CACHED["nc"]
    in_maps = prep_inputs(x, freqs_cis, wq, wk, wv, wo, sinks)
    res = run_bass_kernel_spmd(nc, in_maps, list(range(N_CORES)), trace=False)
    y = np.concatenate([res.results[d]["y"] for d in range(N_CORES)], axis=1)
    return y.reshape(1, S, D).astype(np.float32)
